# revision 38
# baseline (speedup 1.0000x reference)
"""Trainium2 Bass kernel for nn_MultiHeadDuelingDQN (8-core SPMD), v2.

Model (B=256, STATE=26240, H=512, R=4000, N=64 heads, M=10):
    h  = relu(relu(x@W1+b1)@W2+b2)
    q_cache = h@Wvc+bvc + (h@Wac+bac) - mean_R(h@Wac+bac)
    q_assoc = per-head dueling over M (local means)
    q_rec   = S - mean_R(S),  S = sum_n (h@Wru[n]+bru[n])   [exact rewrite:
              rec_global has zero row-mean, so the second mean subtraction
              is a no-op and the [B,N,R] tensor is never formed]

v2 design (vs v1 baseline):
  - Everything staged in bf16 on the wire (weights, x, activations);
    PSUM accumulates fp32; outputs fp32.  Halves the dominant Wru HBM
    stream (65.5 -> 32.8 MB/core).
  - The 64-head sum of Wru (W_sum) is computed on the TensorE via
    identity-matmul PSUM accumulation (64 accumulating matmuls per
    k-chunk) instead of DVE tensor_reduce (which is capped at 1x mode
    and was the 143us bottleneck).  PE does ~210ns per 500-col matmul.
  - x is pre-transposed on the host -> fc1 lhsT tiles load directly
    (no on-chip transposes for x).
  - fc1 partial sums exchanged with ONE AllReduce [256,512] fp32;
    fc2 is then computed replicated on every core (134 MF, ~2us),
    removing the AllGather of h2 entirely.
  - Sharding: fc1 contraction split 8 ways; R split 8 ways for
    cache/rec heads (each core owns 500 columns, reduces over all 64
    heads for its slice -> no big AllReduce of S); 8 assoc heads/core.
  - Row-means over the full R need a tiny [128,4] AllGather at stream
    end (unavoidable sync point).

kernel(**inputs) takes full unsharded inputs, returns full [256, 8640].
"""
import os
os.environ.setdefault("NEURON_RT_DBG_RDH_CC", "0")

import numpy as np
import ml_dtypes

import concourse.bass as bass
import concourse.mybir as mybir
import concourse.tile as tile
from concourse import bacc
from concourse import bass_utils
from concourse.bass import ts
from concourse.masks import make_identity

NC = 8
B, H, STATE, R, NH, M = 256, 512, 26240, 4000, 64, 10
KPC_RAW = STATE // NC          # 3280
KCH = 26                       # k-chunks of 128 per core (padded)
KPC = KCH * 128                # 3328
RPC = R // NC                  # 500
HPC = NH // NC                 # 8 assoc heads per core
AUG = HPC * (M + 1) + 1        # 89 = [8x(10 adv + 1 val)] + value_c
GRP = 8                        # heads per wru DMA tile
NT = NH // GRP                 # 8 tiles per k-chunk (each [128, 8*500] bf16 = 1MB)
W1GROUPS = [(0, 7), (7, 7), (14, 6), (20, 6)]
F32 = mybir.dt.float32
BF16 = mybir.dt.bfloat16
RELU = mybir.ActivationFunctionType.Relu
COPY = mybir.ActivationFunctionType.Copy
IDENT = mybir.ActivationFunctionType.Identity
ADD = mybir.AluOpType.add
BF = ml_dtypes.bfloat16


def build_program():
    nc = bacc.Bacc("TRN2", target_bir_lowering=False, debug=False, num_devices=NC)

    # ---- per-core I/O (all big tensors pre-packed bf16 on host) ----
    xt = nc.dram_tensor("xt", [128, KCH * B], BF16, kind="ExternalInput").ap()
    w1 = nc.dram_tensor("w1", [128, KCH * H], BF16, kind="ExternalInput").ap()
    w2 = nc.dram_tensor("w2", [128, 4 * H], BF16, kind="ExternalInput").ap()
    wac = nc.dram_tensor("wac", [128, 4 * RPC], BF16, kind="ExternalInput").ap()
    waug = nc.dram_tensor("waug", [128, 4 * AUG], BF16, kind="ExternalInput").ap()
    # wru[kc, t, p, gi*500+r] = Wru[t*16+gi, kc*128+p, r0+r]
    wru = nc.dram_tensor("wru", [4, NT, 128, GRP * RPC], BF16,
                         kind="ExternalInput").ap()
    brus = nc.dram_tensor("brus", [NH, RPC], BF16, kind="ExternalInput").ap()
    # biasrow = [b1(/core0 only) | b2 | bac_slice | baug], one row
    NBIAS = H + H + RPC + AUG   # 1613
    biasrow = nc.dram_tensor("biasrow", [1, NBIAS], BF16, kind="ExternalInput").ap()

    out_cache = nc.dram_tensor("out_cache", [B, RPC], F32, kind="ExternalOutput").ap()
    out_rec = nc.dram_tensor("out_rec", [B, RPC], F32, kind="ExternalOutput").ap()
    out_assoc = nc.dram_tensor("out_assoc", [B, HPC * M], F32, kind="ExternalOutput").ap()

    with tile.TileContext(nc) as tc:
        with (
            tc.tile_pool(name="cst", bufs=1) as cst,
            tc.tile_pool(name="sb", bufs=1) as sb,
            tc.tile_pool(name="wrup", bufs=10) as wrup,
            tc.tile_pool(name="psfc", bufs=2, space="PSUM") as psfc,
            tc.tile_pool(name="pssm", bufs=2, space="PSUM") as pssm,
            tc.tile_pool(name="psacc", bufs=1, space="PSUM") as psacc,
            tc.tile_pool(name="pss", bufs=2, space="PSUM") as pss,
            tc.tile_pool(name="psw", bufs=1, space="PSUM") as psw,
            tc.tile_pool(name="dram", bufs=1, space="DRAM") as dram,
        ):
            identB = cst.tile([128, 128], BF16, tag="identB")
            make_identity(nc, identB)
            ones1 = cst.tile([1, 128], BF16, tag="ones1")
            nc.vector.memset(ones1, 1.0)
            ones64 = cst.tile([64, 128], BF16, tag="ones64")
            nc.vector.memset(ones64, 1.0)

            # ---------- DMAs ----------
            # sync queue FIFO: xt, w1 (trunk-critical, full bandwidth first),
            # THEN the 16x2MB wru stream.  scalar queue: small loads +
            # collective bounces + outputs.
            bias_sb = sb.tile([1, NBIAS], BF16, tag="bias_sb")
            nc.scalar.dma_start(bias_sb, biasrow)
            xt_sb = sb.tile([128, KCH * B], BF16, tag="xt_sb")
            nc.scalar.dma_start(xt_sb, xt)
            w1_sb = sb.tile([128, KCH * H], BF16, tag="w1_sb")
            for gi, (base, L) in enumerate(W1GROUPS):
                nc.scalar.dma_start(w1_sb[:, base * H:(base + L) * H],
                                    w1[:, base * H:(base + L) * H])

            # ---------- Wru stream on sync queue (16 x 2MB) ----------
            wt_tiles = []
            for kc in range(4):
                row = []
                for t in range(NT):
                    wt = wrup.tile([128, GRP * RPC], BF16, tag="wru",
                                   name=f"wru_t{kc}_{t}")
                    nc.sync.dma_start(wt, wru[kc, t])
                    row.append(wt)
                wt_tiles.append(row)

            # small loads (scalar queue)
            w2_sb = sb.tile([128, 4 * H], BF16, tag="w2_sb")
            nc.scalar.dma_start(w2_sb, w2)
            wac_sb = sb.tile([128, 4 * RPC], BF16, tag="wac_sb")
            nc.scalar.dma_start(wac_sb, wac)
            waug_sb = sb.tile([128, 4 * AUG], BF16, tag="waug_sb")
            nc.scalar.dma_start(waug_sb, waug)
            brus_sb = sb.tile([NH, RPC], BF16, tag="brus_sb")
            nc.scalar.dma_start(brus_sb, brus)
            b1r = bias_sb[:, 0:H]
            b2r = bias_sb[:, H:2 * H]
            bacr = bias_sb[:, 2 * H:2 * H + RPC]
            baugr = bias_sb[:, 2 * H + RPC:2 * H + RPC + AUG]

            # ---------- PE warm-up ----------
            # ~100 junk matmuls while the xt/w1 DMAs are in flight: keeps the
            # PE HAM activity monitor busy so the clock is at 2.4 GHz (not
            # the 1.2 GHz cold state) when fc1 and the Wru accumulation start.
            for wi in range(100):
                wpt = psfc.tile([128, H], F32, tag="fc", name=f"warm{wi}")
                nc.tensor.matmul(wpt[:, 0:128], identB, identB,
                                 start=True, stop=True)

            # ---------- head-sum of Wru on PE (identity accumulate) ----------
            accSB = [sb.tile([128, RPC], BF16, tag=f"accSB{kc}", name=f"accSB{kc}")
                     for kc in range(4)]
            # g_kc[k] = sum_r accSB[kc][k, r]: feeds the 1-col row-sum
            # matmuls so the final AllGather does not wait on psS/ACT
            g4f = sb.tile([128, 1], F32, tag="g4f")
            g4 = sb.tile([128, 8], BF16, tag="g4")

            def accum_kc(kc):
                psA = psacc.tile([128, RPC], F32, tag="acc", name=f"psACC{kc}")
                for t in range(NT):
                    wt = wt_tiles[kc][t]
                    for gi in range(GRP):
                        nc.tensor.matmul(psA, identB, wt[:, ts(gi, RPC)],
                                         start=(t == 0 and gi == 0),
                                         stop=(t == NT - 1 and gi == GRP - 1))
                nc.vector.tensor_copy(accSB[kc], psA)
                nc.vector.tensor_reduce(g4f, accSB[kc],
                                        axis=mybir.AxisListType.X, op=ADD)
                nc.vector.tensor_copy(g4[:, 2 * kc:2 * kc + 1], g4f)

            accum_kc(0)
            accum_kc(1)

            # ---------- fc1: 52 matmuls into 2 PSUM banks ----------
            h1_ps = [psfc.tile([128, H], F32, tag="fc", name=f"h1_ps{bt}")
                     for bt in range(2)]
            for bt in range(2):
                nc.tensor.matmul(h1_ps[bt], ones1, b1r, start=True, stop=False)
            for base, L in W1GROUPS:
                for j in range(L):
                    kc = base + j
                    for bt in range(2):
                        nc.tensor.matmul(
                            h1_ps[bt],
                            xt_sb[:, kc * B + bt * 128: kc * B + bt * 128 + 128],
                            w1_sb[:, ts(kc, H)],
                            start=False, stop=(kc == KCH - 1))

            # ---------- h1 partial -> DRAM -> AllReduce(add), bf16 ----------
            ar_in = dram.tile([B, H], BF16, tag="ar_in")
            ar_out = dram.tile([B, H], BF16, tag="ar_out")
            for bt in range(2):
                h1c = sb.tile([128, H], BF16, tag=f"h1c{bt}", name=f"h1c{bt}")
                nc.scalar.copy(h1c, h1_ps[bt])
                nc.scalar.dma_start(ar_in[ts(bt, 128), :], h1c)
            nc.gpsimd.collective_compute(
                "AllReduce", ADD,
                replica_groups=[list(range(NC))],
                ins=[ar_in.opt()], outs=[ar_out.opt()],
            )

            accum_kc(2)

            # ---------- trunk post-collective: relu, transpose, fc2 ----------
            h1T = [sb.tile([128, B], BF16, tag=f"h1T{kc}", name=f"h1T{kc}")
                   for kc in range(4)]
            for bt in range(2):
                h1g = sb.tile([128, H], BF16, tag=f"h1g{bt}", name=f"h1g{bt}")
                nc.scalar.dma_start(h1g, ar_out[ts(bt, 128), :])
                h1s = sb.tile([128, H], BF16, tag=f"h1s{bt}", name=f"h1s{bt}")
                nc.scalar.activation(h1s, h1g, RELU)
                for kc in range(4):
                    pt = pssm.tile([128, 128], BF16, tag="small", name=f"pt1_{bt}_{kc}")
                    nc.tensor.transpose(pt, h1s[:, ts(kc, 128)], identB)
                    nc.vector.tensor_copy(h1T[kc][:, ts(bt, 128)], pt)

            hT = [sb.tile([128, B], BF16, tag=f"hT{kc}", name=f"hT{kc}")
                  for kc in range(4)]
            for bt in range(2):
                h2_ps = psfc.tile([128, H], F32, tag="fc", name=f"h2_ps{bt}")
                nc.tensor.matmul(h2_ps, ones1, b2r, start=True, stop=False)
                for kc in range(4):
                    nc.tensor.matmul(h2_ps, h1T[kc][:, ts(bt, 128)],
                                     w2_sb[:, ts(kc, H)],
                                     start=False, stop=(kc == 3))
                h2s = sb.tile([128, H], BF16, tag=f"h2s{bt}", name=f"h2s{bt}")
                nc.scalar.activation(h2s, h2_ps, RELU)
                for kc in range(4):
                    pt = pssm.tile([128, 128], BF16, tag="small", name=f"pt2_{bt}_{kc}")
                    nc.tensor.transpose(pt, h2s[:, ts(kc, 128)], identB)
                    nc.vector.tensor_copy(hT[kc][:, ts(bt, 128)], pt)

            # ---------- assoc heads (augmented [adv|val|value_c]) ----------
            ar2_in = sb.tile([128, 4], F32, tag="ar2_in")
            value_sb = []
            for bt in range(2):
                psA = psw.tile([128, AUG], F32, tag="wide", name=f"psA{bt}")
                nc.tensor.matmul(psA, ones1, baugr, start=True, stop=False)
                for kc in range(4):
                    nc.tensor.matmul(psA, hT[kc][:, ts(bt, 128)],
                                     waug_sb[:, ts(kc, AUG)],
                                     start=False, stop=(kc == 3))
                psA_sb = sb.tile([128, AUG], F32, tag=f"psAsb{bt}", name=f"psAsb{bt}")
                nc.scalar.copy(psA_sb, psA)
                grid = psA_sb[:, 0:HPC * (M + 1)].rearrange("p (n u) -> p n u",
                                                            u=M + 1)
                sums = sb.tile([128, HPC], F32, tag=f"sumsA{bt}", name=f"sumsA{bt}")
                nc.vector.tensor_reduce(sums, grid[:, :, 0:M],
                                        axis=mybir.AxisListType.X, op=ADD)
                negm = sb.tile([128, HPC], F32, tag=f"negmA{bt}", name=f"negmA{bt}")
                nc.vector.tensor_scalar_mul(negm, sums, -1.0 / M)
                tmp = sb.tile([128, HPC], F32, tag=f"tmpA{bt}", name=f"tmpA{bt}")
                nc.gpsimd.tensor_add(tmp, grid[:, :, M], negm)
                q = sb.tile([128, HPC * M], F32, tag=f"qA{bt}", name=f"qA{bt}")
                nc.gpsimd.tensor_tensor(
                    out=q.rearrange("p (n m) -> p n m", m=M),
                    in0=grid[:, :, 0:M],
                    in1=tmp.broadcast_to([128, HPC, M]),
                    op=ADD)
                nc.scalar.dma_start(out_assoc[ts(bt, 128), :], q)
                value_sb.append(psA_sb[:, AUG - 1:AUG])

            # ---------- cache head ----------
            adv_c_sb = []
            for bt in range(2):
                psC = psw.tile([128, RPC], F32, tag="wide", name=f"psC{bt}")
                nc.tensor.matmul(psC, ones1, bacr, start=True, stop=False)
                for kc in range(4):
                    nc.tensor.matmul(psC, hT[kc][:, ts(bt, 128)],
                                     wac_sb[:, ts(kc, RPC)],
                                     start=False, stop=(kc == 3))
                t = sb.tile([128, RPC], F32, tag=f"advc{bt}", name=f"advc{bt}")
                nc.scalar.activation(t, psC, COPY, accum_out=ar2_in[:, bt:bt + 1])
                adv_c_sb.append(t)

            # ---------- S matmuls (interleaved with remaining accum) ----------
            psS = [pss.tile([128, RPC], F32, tag="s", name=f"psS{bt}")
                   for bt in range(2)]
            for bt in range(2):
                nc.tensor.matmul(psS[bt], ones64, brus_sb, start=True, stop=False)
                for kc in range(3):
                    nc.tensor.matmul(psS[bt], hT[kc][:, ts(bt, 128)], accSB[kc],
                                     start=False, stop=False)

            accum_kc(3)
            for bt in range(2):
                nc.tensor.matmul(psS[bt], hT[3][:, ts(bt, 128)], accSB[3],
                                 start=False, stop=True)

            # S row-sums via the g-vector: rowsum_local(S)[b] =
            # sum_kc (hT[kc].T @ g_kc)[b] + colsum_local(bru).  Ten 1-col
            # matmuls (~60ns each) instead of waiting for the psS ACT copy.
            brrf = sb.tile([64, 1], F32, tag="brrf")
            nc.vector.tensor_reduce(brrf, brus_sb, axis=mybir.AxisListType.X,
                                    op=ADD)
            brr = sb.tile([64, 1], BF16, tag="brr")
            nc.vector.tensor_copy(brr, brrf)
            psRS = psw.tile([128, 2], F32, tag="wide", name="psRS")
            for bt in range(2):
                nc.tensor.matmul(psRS[:, bt:bt + 1], ones64, brr,
                                 start=True, stop=False)
                for kc in range(4):
                    nc.tensor.matmul(psRS[:, bt:bt + 1], hT[kc][:, ts(bt, 128)],
                                     g4[:, 2 * kc:2 * kc + 1],
                                     start=False, stop=(kc == 3))
            nc.vector.tensor_copy(ar2_in[:, 2:4], psRS)

            # ---------- tiny AllGather of row-sums ----------
            ar2_din = dram.tile([128, 4], F32, tag="ar2_din")
            ar2_dout = dram.tile([NC * 128, 4], F32, tag="ar2_dout")
            nc.scalar.dma_start(ar2_din, ar2_in)
            nc.gpsimd.collective_compute(
                "AllGather", mybir.AluOpType.bypass,
                replica_groups=[list(range(NC))],
                ins=[ar2_din.opt()], outs=[ar2_dout.opt()],
            )
            # psS -> SBUF copies run concurrently with the collective
            s_sb = []
            for bt in range(2):
                st = sb.tile([128, RPC], F32, tag=f"ssb{bt}", name=f"ssb{bt}")
                nc.scalar.activation(st, psS[bt], COPY)
                s_sb.append(st)
            rall = sb.tile([128, NC * 4], F32, tag="rall")
            nc.scalar.dma_start(rall, ar2_dout.rearrange("(g p) c -> p g c", p=128))
            rview = bass.AP(rall.tensor, rall.offset,
                            [rall.ap[0], [1, 4], [4, NC]])
            ar2_sb = sb.tile([128, 4], F32, tag="ar2_sb")
            nc.vector.tensor_reduce(ar2_sb, rview, axis=mybir.AxisListType.X, op=ADD)
            negmeans = sb.tile([128, 4], F32, tag="negmeans")
            nc.scalar.activation(negmeans, ar2_sb, COPY, scale=-1.0 / R)

            for bt in range(2):
                vm = sb.tile([128, 1], F32, tag=f"vm{bt}", name=f"vm{bt}")
                nc.gpsimd.tensor_add(vm, value_sb[bt], negmeans[:, bt:bt + 1])
                qc = sb.tile([128, RPC], F32, tag=f"qc{bt}", name=f"qc{bt}")
                nc.scalar.activation(qc, adv_c_sb[bt], IDENT, bias=vm, scale=1.0)
                nc.scalar.dma_start(out_cache[ts(bt, 128), :], qc)

                qr = sb.tile([128, RPC], F32, tag=f"qr{bt}", name=f"qr{bt}")
                nc.scalar.activation(qr, s_sb[bt], IDENT,
                                     bias=negmeans[:, 2 + bt:3 + bt], scale=1.0)
                nc.scalar.dma_start(out_rec[ts(bt, 128), :], qr)

    nc.compile()
    return nc


_CACHED = None


def _get_program():
    global _CACHED
    if _CACHED is None:
        _CACHED = build_program()
    return _CACHED


def make_in_maps(x, W1, b1, W2, b2, Wvc, bvc, Wac, bac, Wvu, bvu, Wau, bau, Wru, bru):
    f = np.float32
    x16 = np.asarray(x, f).astype(BF)                    # [256, 26240]
    W1_16 = np.asarray(W1, f).astype(BF)                 # [26240, 512]
    W2_16 = np.asarray(W2, f).astype(BF)                 # [512, 512]
    Wac16 = np.asarray(Wac, f).astype(BF)                # [512, 4000]
    Wau16 = np.asarray(Wau, f).astype(BF)                # [64, 512, 10]
    Wvu16 = np.asarray(Wvu, f).astype(BF)                # [64, 512]
    Wvc16 = np.asarray(Wvc, f).astype(BF).reshape(H)     # [512]
    Wru16 = np.asarray(Wru, f).astype(BF)                # [64, 512, 4000]
    b1_16 = np.asarray(b1, f).astype(BF)
    b2_16 = np.asarray(b2, f).astype(BF)
    bac16 = np.asarray(bac, f).astype(BF)
    bau16 = np.asarray(bau, f).astype(BF)                # [64, 10]
    bvu16 = np.asarray(bvu, f).astype(BF)                # [64]
    bvc16 = np.asarray(bvc, f).astype(BF).reshape(1)
    bru16 = np.asarray(bru, f).astype(BF)                # [64, 4000]

    # w2 pack [128, 4*512]: w2p[p, kc*512+h] = W2[kc*128+p, h]
    w2p = np.ascontiguousarray(
        W2_16.reshape(4, 128, H).transpose(1, 0, 2)).reshape(128, 4 * H)

    # wru pack for all cores at once:
    # wru_c[kc, t, p, gi*500+r] = Wru[t*16+gi, kc*128+p, c*500+r]
    A = Wru16.reshape(NT, GRP, 4, 128, NC, RPC)          # [t, gi, kc, p, c, r]
    A = np.ascontiguousarray(A.transpose(4, 2, 0, 3, 1, 5))  # [c, kc, t, p, gi, r]
    wru_all = A.reshape(NC, 4, NT, 128, GRP * RPC)

    in_maps = []
    for c in range(NC):
        k0 = c * KPC_RAW
        # xt pack [128, 26*256]: xt[p, kc*256+b] = x[b, k0+kc*128+p]
        xs = np.zeros((KPC, B), BF)
        xs[:KPC_RAW] = x16[:, k0:k0 + KPC_RAW].T
        xtp = np.ascontiguousarray(
            xs.reshape(KCH, 128, B).transpose(1, 0, 2)).reshape(128, KCH * B)
        # w1 pack [128, 26*512]
        w1s = np.zeros((KPC, H), BF)
        w1s[:KPC_RAW] = W1_16[k0:k0 + KPC_RAW]
        w1p = np.ascontiguousarray(
            w1s.reshape(KCH, 128, H).transpose(1, 0, 2)).reshape(128, KCH * H)

        r0 = c * RPC
        h0 = c * HPC
        # wac pack [128, 4*500]
        wacp = np.ascontiguousarray(
            Wac16[:, r0:r0 + RPC].reshape(4, 128, RPC).transpose(1, 0, 2)
        ).reshape(128, 4 * RPC)
        # waug pack [128, 4*89]: per kc: [8 heads x (10 adv + 1 val)] + wvc
        waugp = np.zeros((128, 4, AUG), BF)
        wau_c = Wau16[h0:h0 + HPC]                       # [8, 512, 10]
        wvu_c = Wvu16[h0:h0 + HPC]                       # [8, 512]
        for kc in range(4):
            sl = slice(kc * 128, (kc + 1) * 128)
            blk = waugp[:, kc, :HPC * (M + 1)].reshape(128, HPC, M + 1)
            blk[:, :, :M] = wau_c[:, sl, :].transpose(1, 0, 2)
            blk[:, :, M] = wvu_c[:, sl].T
            waugp[:, kc, AUG - 1] = Wvc16[sl]
        waugp = np.ascontiguousarray(waugp).reshape(128, 4 * AUG)

        # biasrow [1, 1613]: [b1(/core0) | b2 | bac slice | baug]
        brow = np.zeros((1, 2 * H + RPC + AUG), BF)
        if c == 0:
            brow[0, :H] = b1_16
        brow[0, H:2 * H] = b2_16
        brow[0, 2 * H:2 * H + RPC] = bac16[r0:r0 + RPC]
        baug = np.zeros(AUG, BF)
        bb = baug[:HPC * (M + 1)].reshape(HPC, M + 1)
        bb[:, :M] = bau16[h0:h0 + HPC]
        bb[:, M] = bvu16[h0:h0 + HPC]
        baug[AUG - 1] = bvc16[0]
        brow[0, 2 * H + RPC:] = baug

        m = {
            "xt": xtp,
            "w1": w1p,
            "w2": w2p,
            "wac": wacp,
            "waug": waugp,
            "wru": wru_all[c],
            "brus": np.ascontiguousarray(bru16[:, r0:r0 + RPC]),
            "biasrow": brow,
        }
        in_maps.append(m)
    return in_maps


def assemble(results):
    q = np.empty((B, 2 * R + NH * M), np.float32)
    for c in range(NC):
        r0 = c * RPC
        a0 = c * HPC * M
        q[:, r0:r0 + RPC] = results[c]["out_cache"]
        q[:, R + r0:R + r0 + RPC] = results[c]["out_rec"]
        q[:, 2 * R + a0:2 * R + a0 + HPC * M] = results[c]["out_assoc"]
    return q


def run(in_maps, **kw):
    nc = _get_program()
    return bass_utils.run_bass_kernel_spmd(nc, in_maps, core_ids=list(range(NC)), **kw)


def kernel(**inputs):
    in_maps = make_in_maps(**{k: np.asarray(v) for k, v in inputs.items()})
    res = run(in_maps)
    return assemble(res.results)
